# revision 1
# baseline (speedup 1.0000x reference)
"""CantorAttention Trainium2 kernel (8 NeuronCores).

Architecture
------------
The reference gathers K=64 routed keys/values per query (with +-1 smoothing)
and does sparse attention. Gathering k/v rows on TRN2 is bandwidth-doomed
(gathered tensor is 256MB); instead we *rematerialize densely* on the PE:

  smoothing commutes with the gather:  k_g[s,i] = k_s[r[s,i]] where
  k_s[j] = 0.5*k[j] + 0.25*(k[max(j-1,0)] + k[min(j+1,S-1)])

  softmax over 64 slots (with duplicate routes) == dense masked softmax with
  multiplicity weights M[s,j] = #{i : r[s,i] = j}:

    out[s] = sum_j M[s,j] * exp(zd[s,j]) * v_s[j] / sum_j M[s,j] * exp(zd[s,j])
    zd[s,j] = scale * q[s] . k_s[j]

so everything becomes dense matmuls + one dense exp + one dense mask-multiply.

Sharding: phase 1 = one head per core (scores/softmax/AV, outputs
unnormalized head outputs + softmax denominators); phase 2 = output
projection, sharded over sequence (each core takes 256 query positions,
all heads), avoiding any on-device collective.

Layout notes: phase-1 keeps everything transposed ([dim, seq] /
[key-block, seq]) so the sequence axis is always the matmul moving dim and
the smoothing shift is a free-dim offset. exp() runs with no max-subtract:
zd = q.k_s/8 with ~N(0,1) entries, |zd| << 80, so fp32/bf16 exp is safe.
"""
import sys

sys.path.insert(0, "/opt/trn_rl_repo")

import numpy as np
import ml_dtypes

import concourse.bass as bass
import concourse.bacc as bacc
import concourse.mybir as mybir
from concourse import tile
from concourse import bass_utils

BF16 = mybir.dt.bfloat16
F32 = mybir.dt.float32
Exp = mybir.ActivationFunctionType.Exp
Copy = mybir.ActivationFunctionType.Copy
ADD = mybir.AluOpType.add
MULT = mybir.AluOpType.mult

S = 2048  # sequence length
D = 512  # model dim
H = 8  # heads
HD = 64  # head dim
KN = 64  # routed neighbors per query
NCORES = 8
SC = S // 512  # moving-dim chunks of 512
JB = S // 128  # key blocks of 128

_nc1 = None
_nc2 = None


def _build_phase1():
    nc = bacc.Bacc("TRN2", target_bir_lowering=False, debug=False, num_devices=NCORES)
    xt_d = nc.dram_tensor("xt", [128, 4 * S], BF16, kind="ExternalInput").ap()
    wq_d = nc.dram_tensor("wq", [128, 4 * HD], BF16, kind="ExternalInput").ap()
    wkv_d = nc.dram_tensor("wkv", [128, 4 * 2 * HD], BF16, kind="ExternalInput").ap()
    bq_d = nc.dram_tensor("bq", [HD, 1], F32, kind="ExternalInput").ap()
    bkv_d = nc.dram_tensor("bkv", [2 * HD, 1], F32, kind="ExternalInput").ap()
    mt_d = nc.dram_tensor("mt", [S, S], BF16, kind="ExternalInput").ap()
    id64_d = nc.dram_tensor("id64", [128, HD], BF16, kind="ExternalInput").ap()
    outu_d = nc.dram_tensor("outu", [HD + 1, S], F32, kind="ExternalOutput").ap()

    with tile.TileContext(nc) as tc:
        with (
            tc.tile_pool(name="const", bufs=1) as const,
            tc.tile_pool(name="work", bufs=1) as work,
            tc.tile_pool(name="mstream", bufs=6) as mstream,
            tc.tile_pool(name="estream", bufs=4) as estream,
            tc.tile_pool(name="ps_big", bufs=1, space="PSUM") as ps_big,
        ):
            xt = const.tile([128, 4 * S], BF16)
            wq = const.tile([128, 4 * HD], BF16)
            wkv = const.tile([128, 4 * 2 * HD], BF16)
            bq = const.tile([HD, 1], F32)
            bkv = const.tile([2 * HD, 1], F32)
            id64 = const.tile([128, HD], BF16)
            nc.sync.dma_start(wkv[:], wkv_d[:])
            nc.sync.dma_start(wq[:], wq_d[:])
            nc.sync.dma_start(bq[:], bq_d[:])
            nc.sync.dma_start(bkv[:], bkv_d[:])
            nc.sync.dma_start(id64[:], id64_d[:])
            for c in range(4):
                nc.sync.dma_start(
                    xt[:, 2048 * c : 2048 * (c + 1)], xt_d[:, 2048 * c : 2048 * (c + 1)]
                )

            qt = work.tile([128, S], BF16)  # q^T * (1/16); rows 64-127 = copy
            kx = work.tile([128, S], BF16)  # rows 64-127 = copy of k~^T
            kvpad = work.tile([128, S + 2], F32)  # rows 0-63 k^T, 64-127 v^T
            kvs = work.tile([128, S], BF16)  # smoothed k~^T / v~^T
            vaug = work.tile([128, JB * 128], BF16)  # v~ blocks + ones col (128-stride aligned)

            # PSUM: 8 banks. Two half-width zd tiles (2 banks each) double-
            # buffer the scores->exp pipeline; outu (4 banks) accumulates AV.
            # All are also reused as projection accumulators via slices.
            zd_a = ps_big.tile([128, S // 2], F32)
            zd_b = ps_big.tile([128, S // 2], F32)
            outu_ps = ps_big.tile([HD + 1, S], F32)

            # --- kv projection (transposed): psum[c,s] = sum_d W[d,c] xT[d,s]
            # biases are folded into the PSUM->SBUF copies (per-partition adds)
            for sc in range(SC):
                kv_ps = (zd_a if sc % 2 == 0 else zd_b)[:, 512 * (sc // 2) : 512 * (sc // 2 + 1)]
                for c in range(4):
                    rhs = xt[:, 2048 * c + 512 * sc : 2048 * c + 512 * (sc + 1)]
                    nc.tensor.matmul(
                        kv_ps, wkv[:, 128 * c : 128 * (c + 1)], rhs, start=(c == 0), stop=(c == 3)
                    )
                nc.vector.tensor_scalar_add(
                    kvpad[:, 1 + 512 * sc : 1 + 512 * (sc + 1)], kv_ps, bkv[:]
                )

            # smoothing leads the DVE queue (ahead of the q bias-copies) so
            # the transposes it gates start ASAP; the q projection matmuls
            # fill the PE meanwhile, keeping the HAM clock-gate warm.
            nc.vector.tensor_copy(kvpad[:, 0:1], kvpad[:, 1:2])
            nc.vector.tensor_copy(kvpad[:, S + 1 : S + 2], kvpad[:, S : S + 1])
            tsm = work.tile([128, S], F32)
            # t = 0.5*shiftL + base ; kvs = 0.5*shiftR + t   (kvs = 2 * smoothed)
            nc.vector.scalar_tensor_tensor(
                tsm[:], kvpad[:, 0:S], 0.5, kvpad[:, 1 : S + 1], MULT, ADD
            )
            nc.vector.scalar_tensor_tensor(
                kvs[:], kvpad[:, 2 : S + 2], 0.5, tsm[:], MULT, ADD
            )

            for sc in range(SC):
                q_ps = outu_ps[0:HD, 512 * sc : 512 * (sc + 1)]
                for c in range(4):
                    rhs = xt[:, 2048 * c + 512 * sc : 2048 * c + 512 * (sc + 1)]
                    nc.tensor.matmul(
                        q_ps, wq[:, HD * c : HD * (c + 1)], rhs, start=(c == 0), stop=(c == 3)
                    )
                nc.vector.tensor_scalar_add(qt[0:HD, 512 * sc : 512 * (sc + 1)], q_ps, bq[:])

            # duplicate k~^T and q^T into partitions 64-127 so score matmuls
            # for odd key-blocks can run in PE row-group 64-127 concurrently
            # with even key-blocks in rows 0-63 (row-packed pairs)
            nc.scalar.dma_start(kx[HD:128, :], kvs[0:HD, :])
            nc.scalar.dma_start(qt[HD:128, :], qt[0:HD, :])

            # --- v~ blocks transposed into [j-in-block, hd] layout + ones col.
            # PE-mode transpose (not DMA): keeps the PE busy and off the DMA
            # queues; outputs staged through the (currently free) zd PSUM.
            for jb in range(JB):
                tp = (zd_a if jb % 2 == 0 else zd_b)[
                    :, 64 * (jb // 2) : 64 * (jb // 2) + HD
                ].bitcast(BF16)[:, 0:HD]
                nc.tensor.transpose(tp, kvs[HD:128, 128 * jb : 128 * (jb + 1)], id64[HD:128, :])
                nc.vector.tensor_copy(vaug[:, 128 * jb : 128 * jb + HD], tp)
                nc.gpsimd.memset(vaug[:, 128 * jb + HD : 128 * jb + HD + 1], 1.0)

            # --- dense masked attention: key-block PAIRS x seq-halves.
            # jb0 scores run in PE rows 0-63 while jb1 runs rows 64-127
            # (row-group packing); exp(zd_a) frees zd_a while exp(zd_b) and
            # the AV matmuls still overlap the next pair's scores.
            H2 = S // 2
            for u in range(JB):
                jp, sh = u // 2, u % 2
                jb0, jb1 = 2 * jp, 2 * jp + 1
                for c in range(2):
                    nc.tensor.matmul(
                        zd_a[:, 512 * c : 512 * (c + 1)],
                        kvs[0:HD, 128 * jb0 : 128 * (jb0 + 1)],
                        qt[0:HD, H2 * sh + 512 * c : H2 * sh + 512 * (c + 1)],
                        start=True,
                        stop=True,
                    )
                    nc.tensor.matmul(
                        zd_b[:, 512 * c : 512 * (c + 1)],
                        kx[HD:128, 128 * jb1 : 128 * (jb1 + 1)],
                        qt[HD:128, H2 * sh + 512 * c : H2 * sh + 512 * (c + 1)],
                        start=True,
                        stop=True,
                    )
                for half, (zd, jb) in enumerate(((zd_a, jb0), (zd_b, jb1))):
                    e_bf = estream.tile([128, H2], BF16, tag="e")
                    nc.scalar.activation(e_bf[:], zd[:], Exp)
                    m_bf = mstream.tile([128, H2], BF16, tag="m")
                    nc.sync.dma_start(
                        m_bf[:], mt_d[128 * jb : 128 * (jb + 1), H2 * sh : H2 * (sh + 1)]
                    )
                    nc.vector.tensor_mul(e_bf[:], e_bf[:], m_bf[:])
                    for c in range(2):
                        nc.tensor.matmul(
                            outu_ps[:, H2 * sh + 512 * c : H2 * sh + 512 * (c + 1)],
                            vaug[:, 128 * jb : 128 * jb + HD + 1],
                            e_bf[:, 512 * c : 512 * (c + 1)],
                            start=(jb == 0),
                            stop=(jb == JB - 1),
                        )
            outu_sb = work.tile([HD + 1, S], F32)
            for c in range(SC):
                nc.vector.tensor_copy(
                    outu_sb[:, 512 * c : 512 * (c + 1)], outu_ps[:, 512 * c : 512 * (c + 1)]
                )
                eng = nc.sync if c % 2 == 0 else nc.scalar
                eng.dma_start(
                    outu_d[:, 512 * c : 512 * (c + 1)], outu_sb[:, 512 * c : 512 * (c + 1)]
                )
    nc.compile()
    return nc


def _build_phase2():
    nc = bacc.Bacc("TRN2", target_bir_lowering=False, debug=False, num_devices=NCORES)
    SS = S // NCORES  # 256 query positions per core
    u_d = nc.dram_tensor("u", [128, 4 * SS], F32, kind="ExternalInput").ap()
    l_d = nc.dram_tensor("l", [H, SS], F32, kind="ExternalInput").ap()
    wo_d = nc.dram_tensor("wo", [128, 4 * D], BF16, kind="ExternalInput").ap()
    bo_d = nc.dram_tensor("bo", [1, D], BF16, kind="ExternalInput").ap()
    bl_d = nc.dram_tensor("bl", [H, D], F32, kind="ExternalInput").ap()
    y_d = nc.dram_tensor("y", [SS, D], F32, kind="ExternalOutput").ap()

    with tile.TileContext(nc) as tc:
        with (
            tc.tile_pool(name="sb", bufs=1) as sb,
            tc.tile_pool(name="ps", bufs=2, space="PSUM") as ps,
        ):
            u = sb.tile([128, 4 * SS], F32)
            lt = sb.tile([H, SS], F32)
            wo = sb.tile([128, 4 * D], BF16)
            bo = sb.tile([1, D], BF16)
            bl = sb.tile([H, D], F32)
            onescol = sb.tile([1, 128], BF16)
            nc.sync.dma_start(lt[:], l_d[:])
            nc.sync.dma_start(bl[:], bl_d[:])
            nc.sync.dma_start(bo[:], bo_d[:])
            for c in range(4):
                nc.sync.dma_start(u[:, SS * c : SS * (c + 1)], u_d[:, SS * c : SS * (c + 1)])
                nc.scalar.dma_start(wo[:, D * c : D * (c + 1)], wo_d[:, D * c : D * (c + 1)])
            nc.gpsimd.memset(onescol[:], 1.0)

            rl = sb.tile([H, SS], F32)
            nc.vector.reciprocal(rl[:], lt[:])

            # broadcast 1/l to all 64 rows of each head block: rl_ps[r, s]
            rl_ps = ps.tile([128, 4 * SS], F32)
            for c in range(4):
                nc.tensor.matmul(
                    rl_ps[:, SS * c : SS * (c + 1)],
                    bl[:, 128 * c : 128 * (c + 1)],
                    rl[:],
                    start=True,
                    stop=True,
                )
            un = sb.tile([128, 4 * SS], BF16)
            for c in range(4):
                nc.vector.tensor_mul(
                    un[:, SS * c : SS * (c + 1)],
                    u[:, SS * c : SS * (c + 1)],
                    rl_ps[:, SS * c : SS * (c + 1)],
                )

            for sb2 in range(SS // 128):
                y_ps = ps.tile([128, D], F32, tag="yps")
                for c in range(4):
                    nc.tensor.matmul(
                        y_ps[:],
                        un[:, SS * c + 128 * sb2 : SS * c + 128 * (sb2 + 1)],
                        wo[:, D * c : D * (c + 1)],
                        start=(c == 0),
                        stop=False,
                    )
                nc.tensor.matmul(y_ps[:], onescol[:], bo[:], start=False, stop=True)
                y_sb = sb.tile([128, D], F32, tag="ysb")
                nc.vector.tensor_copy(y_sb[:], y_ps[:])
                nc.sync.dma_start(y_d[128 * sb2 : 128 * (sb2 + 1), :], y_sb[:])
    nc.compile()
    return nc


def _prep_phase1_inputs(x, routes, W_qkv, b_qkv):
    x2 = np.asarray(x, dtype=np.float32).reshape(S, D)
    xt = np.ascontiguousarray(x2.T)  # [D, S]
    xt_r = (
        xt.reshape(4, 128, S).transpose(1, 0, 2).reshape(128, 4 * S).astype(ml_dtypes.bfloat16)
    )
    W = np.asarray(W_qkv, dtype=np.float32)
    b = np.asarray(b_qkv, dtype=np.float32)
    r = np.asarray(routes)
    M = np.zeros((S, S), dtype=np.float32)
    np.add.at(M, (np.arange(S)[:, None], r), 1.0)
    mt = np.ascontiguousarray(M.T).astype(ml_dtypes.bfloat16)

    idf = np.zeros((128, HD), dtype=ml_dtypes.bfloat16)
    idf[HD:128, :] = np.eye(HD, dtype=ml_dtypes.bfloat16)
    in_maps = []
    for h in range(NCORES):
        wq = W[:, h * HD : (h + 1) * HD] * (1.0 / 16.0)
        wk = W[:, D + h * HD : D + (h + 1) * HD]
        wv = W[:, 2 * D + h * HD : 2 * D + (h + 1) * HD]
        wkv = np.concatenate([wk, wv], axis=1)  # [D, 128]
        bq = b[h * HD : (h + 1) * HD] * (1.0 / 16.0)
        bkv = np.concatenate(
            [b[D + h * HD : D + (h + 1) * HD], b[2 * D + h * HD : 2 * D + (h + 1) * HD]]
        )
        in_maps.append(
            {
                "xt": xt_r,
                "wq": wq.reshape(4, 128, HD).transpose(1, 0, 2).reshape(128, 4 * HD).astype(ml_dtypes.bfloat16),
                "wkv": wkv.reshape(4, 128, 2 * HD).transpose(1, 0, 2).reshape(128, 8 * HD).astype(ml_dtypes.bfloat16),
                "bq": np.ascontiguousarray(bq.reshape(HD, 1), dtype=np.float32),
                "bkv": np.ascontiguousarray(bkv.reshape(2 * HD, 1), dtype=np.float32),
                "mt": mt,
                "id64": idf,
            }
        )
    return in_maps


def _prep_phase2_inputs(outs, W_out, b_out):
    SS = S // NCORES
    U = np.concatenate([o[0:HD, :] for o in outs], axis=0)  # [512, S] f32
    L = np.stack([o[HD, :] for o in outs], axis=0)  # [8, S]
    wo = (0.5 * np.asarray(W_out, dtype=np.float32)).astype(ml_dtypes.bfloat16)
    wo_r = np.ascontiguousarray(wo).reshape(4, 128, D).transpose(1, 0, 2).reshape(128, 4 * D)
    bo = np.asarray(b_out, dtype=np.float32).reshape(1, D).astype(ml_dtypes.bfloat16)
    bl = np.zeros((H, D), dtype=np.float32)
    for h in range(H):
        bl[h, h * HD : (h + 1) * HD] = 1.0
    in_maps = []
    for c in range(NCORES):
        Uc = U[:, c * SS : (c + 1) * SS]
        u_r = np.ascontiguousarray(Uc).reshape(4, 128, SS).transpose(1, 0, 2).reshape(128, 4 * SS)
        in_maps.append(
            {
                "u": np.ascontiguousarray(u_r),
                "l": np.ascontiguousarray(L[:, c * SS : (c + 1) * SS]),
                "wo": np.ascontiguousarray(wo_r),
                "bo": bo,
                "bl": bl,
            }
        )
    return in_maps


def _run(nc, in_maps, **kw):
    return bass_utils.run_bass_kernel_spmd(nc, in_maps, list(range(NCORES)), **kw)


def kernel(x, routes, W_qkv, b_qkv, W_out, b_out, _timing=None):
    global _nc1, _nc2
    if _nc1 is None:
        _nc1 = _build_phase1()
    if _nc2 is None:
        _nc2 = _build_phase2()

    in1 = _prep_phase1_inputs(x, routes, W_qkv, b_qkv)
    r1 = _run(_nc1, in1)
    outs = [r1.results[h]["outu"] for h in range(NCORES)]

    in2 = _prep_phase2_inputs(outs, W_out, b_out)
    r2 = _run(_nc2, in2)
    SS = S // NCORES
    y = np.concatenate([r2.results[c]["y"] for c in range(NCORES)], axis=0)

    if _timing is not None:
        _timing["r1"] = r1
        _timing["r2"] = r2
        _timing["in1"] = in1
        _timing["in2"] = in2
    return y.reshape(1, S, D).astype(np.float32)



# revision 9
# speedup vs baseline: 1.9483x; 1.9483x over previous
"""CantorAttention Trainium2 kernel (8 NeuronCores) — single-NEFF block-sparse.

Key ideas
---------
1. Sorting positions by Cantor coordinate makes each query's 64 routed keys
   fall in a narrow contiguous window of the sorted order (the k-nearest
   neighbours in 1D are contiguous after sorting). Sharding the *sorted*
   sequence 8 ways gives each core 256 queries whose keys live in a 384-wide
   rank window -> 3 key blocks of 128 instead of 16 (5.3x less attention
   work than dense-masked attention).
2. The +-1 position smoothing commutes with the (linear) qkv projection, so
   the host feeds x-smoothed (x~) gathered in rank order and the device
   projects it directly into smoothed k~/v~ — no on-device gather, no
   neighbour shifts.
3. Softmax over routed slots == dense masked softmax over the window with
   multiplicity mask M (duplicate-free routes -> 0/1), evaluated as
   exp(scores) * M with the denominator from an appended ones-column in the
   AV matmul.
4. Each core computes *all heads* for its query slice, so the output
   projection is local: one NEFF total, no cross-core exchange, no second
   launch overhead.

Layout: everything stays transposed ([channel, seq]) so seq is the moving
dim. Head *pairs* share 128-partition tiles (even head rows 0:64, odd rows
64:128) letting score matmuls run as concurrent PE row groups.
"""
import sys

sys.path.insert(0, "/opt/trn_rl_repo")

import numpy as np
import ml_dtypes

import concourse.bass as bass
import concourse.bacc as bacc
import concourse.mybir as mybir
from concourse import tile
from concourse import bass_utils

BF16 = mybir.dt.bfloat16
F32 = mybir.dt.float32
Exp = mybir.ActivationFunctionType.Exp
Copy = mybir.ActivationFunctionType.Copy
Identity = mybir.ActivationFunctionType.Identity

S = 2048
D = 512
H = 8
HD = 64
NCORES = 8
Q = S // NCORES  # 256 queries per core
QH = Q // 2      # query half (pipelined)

_nc_cache = {}


def _cantor_coords(seq_len, depth=8):
    x = np.arange(seq_len, dtype=np.float64) / max(1, seq_len - 1)
    x = np.clip(x, 1e-06, 1.0 - 1e-06)
    c = np.zeros_like(x)
    factor = 0.5
    for _ in range(depth):
        xs = x * 3.0
        digit = xs.astype(np.int64)
        x = xs - digit
        c = c + (digit == 2).astype(np.float64) * factor
        factor *= 0.5
    return np.clip(c, 0.0, 1.0)


def _candidate_orders(routes):
    Sl = routes.shape[0]
    yield np.argsort(_cantor_coords(Sl), kind="stable")
    try:
        import scipy.sparse as sp
        from scipy.sparse.csgraph import reverse_cuthill_mckee

        rows = np.repeat(np.arange(Sl), routes.shape[1])
        cols = np.asarray(routes).ravel()
        A = sp.coo_matrix(
            (np.ones(rows.size, dtype=np.float32), (rows, cols)), shape=(Sl, Sl)
        ).tocsr()
        yield np.asarray(reverse_cuthill_mckee(A + A.T)).astype(np.int64)
    except Exception:
        pass
    yield np.arange(Sl)


def _plan(routes):
    """Pick ordering + window width. Returns (perm, rank, w0s, W)."""
    routes = np.asarray(routes).astype(np.int64)
    best = None
    for perm in _candidate_orders(routes):
        rank = np.empty(S, dtype=np.int64)
        rank[perm] = np.arange(S)
        lo = np.empty(NCORES, dtype=np.int64)
        hi = np.empty(NCORES, dtype=np.int64)
        for c in range(NCORES):
            kr = rank[routes[perm[Q * c : Q * (c + 1)]]]
            lo[c], hi[c] = kr.min(), kr.max()
        width = int((hi - lo + 1).max())
        if best is None or width < best[0]:
            best = (width, perm, rank, lo)
        if width <= 384:
            break
    width, perm, rank, lo = best
    W = 384
    while W < width:
        W += 128
    W = min(W, S)
    w0s = np.minimum(np.maximum(lo, 0), S - W)
    return perm, rank, w0s, W


def _build(W):
    """Single-NEFF kernel for window width W (multiple of 128)."""
    NB = W // 128
    nc = bacc.Bacc("TRN2", target_bir_lowering=False, debug=False, num_devices=NCORES)
    xq_d = nc.dram_tensor("xq", [128, 4 * Q], BF16, kind="ExternalInput").ap()
    xw_d = nc.dram_tensor("xw", [128, 4 * W], BF16, kind="ExternalInput").ap()
    wq_d = nc.dram_tensor("wq", [128, 4 * D], BF16, kind="ExternalInput").ap()
    wkv_d = nc.dram_tensor("wkv", [128, 4 * 2 * D], BF16, kind="ExternalInput").ap()
    wo_d = nc.dram_tensor("wo", [HD, 8 * D], BF16, kind="ExternalInput").ap()
    mt_d = nc.dram_tensor("mt", [128, 2 * W], BF16, kind="ExternalInput").ap()
    bq_d = nc.dram_tensor("bq", [128, 4], F32, kind="ExternalInput").ap()
    bkv_d = nc.dram_tensor("bkv", [128, 8], F32, kind="ExternalInput").ap()
    bo_d = nc.dram_tensor("bo", [128, 4], F32, kind="ExternalInput").ap()
    id2_d = nc.dram_tensor("id2", [128, 128], BF16, kind="ExternalInput").ap()
    ones_d = nc.dram_tensor("ones", [128, HD], BF16, kind="ExternalInput").ap()
    y_d = nc.dram_tensor("y", [128, 2 * 4 * QH], F32, kind="ExternalOutput").ap()

    with tile.TileContext(nc) as tc:
        with (
            tc.tile_pool(name="const", bufs=1) as const,
            tc.tile_pool(name="work", bufs=1) as work,
            tc.tile_pool(name="estream", bufs=4) as estream,
            tc.tile_pool(name="ps", bufs=1, space="PSUM") as ps,
        ):
            # ---- constants / inputs ------------------------------------
            xq = const.tile([128, 4 * Q], BF16)
            xw = const.tile([128, 4 * W], BF16)
            wq = const.tile([128, 4 * D], BF16)
            wkv = const.tile([128, 4 * 2 * D], BF16)
            wo = const.tile([HD, 8 * D], BF16)
            mt = const.tile([128, 2 * W], BF16)
            bq = const.tile([128, 4], F32)
            bkv = const.tile([128, 8], F32)
            bo = const.tile([128, 4], F32)
            id2 = const.tile([128, 128], BF16)
            ones = const.tile([128, HD], BF16)

            # critical path: qproj needs wq+xq, then kvproj needs wkv+xw.
            for c in range(4):
                nc.sync.dma_start(wq[:, D * c : D * (c + 1)], wq_d[:, D * c : D * (c + 1)])
                nc.scalar.dma_start(xq[:, Q * c : Q * (c + 1)], xq_d[:, Q * c : Q * (c + 1)])
            for c in range(4):
                nc.sync.dma_start(
                    wkv[:, 2 * D * c : 2 * D * (c + 1)], wkv_d[:, 2 * D * c : 2 * D * (c + 1)]
                )
                nc.scalar.dma_start(xw[:, W * c : W * (c + 1)], xw_d[:, W * c : W * (c + 1)])
            nc.gpsimd.dma_start(bq[:], bq_d[:])
            nc.gpsimd.dma_start(bkv[:], bkv_d[:])
            nc.gpsimd.dma_start(bo[:], bo_d[:])
            nc.gpsimd.dma_start(id2[:], id2_d[:])
            nc.gpsimd.dma_start(ones[:], ones_d[:])
            nc.scalar.dma_start(mt[:], mt_d[:])
            nc.gpsimd.dma_start(wo[:], wo_d[:])

            # ---- persistent SBUF ---------------------------------------
            qt = work.tile([128, 4 * Q], BF16)       # pair j: rows 0:64=q2j, 64:128=q2j+1
            kts = [work.tile([128, W], BF16, name=f"kt{j}") for j in range(4)]
            vts = [work.tile([128, W], BF16, name=f"vt{j}") for j in range(4)]
            # vaug[j][b]: [key, v_even(64) | v_odd(64)]
            vaug = [
                [work.tile([128, 128], BF16, name=f"va{j}_{b}") for b in range(NB)]
                for j in range(4)
            ]
            un = [work.tile([HD, H * QH], BF16, name=f"un{t}") for t in range(2)]
            rcp = [work.tile([HD, H * QH], F32, name=f"rcp{t}") for t in range(2)]
            y_sb = work.tile([128, 2 * 4 * QH], F32)

            # ---- projections -------------------------------------------
            # q: pair j -> psum [128, Q]; evict to qt with bias
            for j in range(4):
                qp = ps.tile([128, 512], F32, tag="big", bufs=2, name=f"qp{j}")
                for c in range(4):
                    nc.tensor.matmul(
                        qp[:, 0:Q],
                        wq[:, D * c + 128 * j : D * c + 128 * (j + 1)],
                        xq[:, Q * c : Q * (c + 1)],
                        start=(c == 0),
                        stop=(c == 3),
                    )
                nc.scalar.activation(
                    qt[:, Q * j : Q * (j + 1)], qp[:, 0:Q], Identity, bias=bq[:, j : j + 1]
                )
            # kv: group g (k-pair j at g=2j, v-pair j at g=2j+1)
            for j in range(4):
                for t, dst in ((0, kts[j]), (1, vts[j])):
                    g = 2 * j + t
                    col = 128 * j if t == 0 else D + 128 * j
                    kp = ps.tile([128, 512], F32, tag="big", bufs=2, name=f"kp{g}")
                    for c in range(4):
                        nc.tensor.matmul(
                            kp[:, 0:W],
                            wkv[:, 2 * D * c + col : 2 * D * c + col + 128],
                            xw[:, W * c : W * (c + 1)],
                            start=(c == 0),
                            stop=(c == 3),
                        )
                    bcol = j if t == 0 else 4 + j
                    nc.scalar.activation(
                        dst[:], kp[:, 0:W], Identity, bias=bkv[:, bcol : bcol + 1]
                    )

            # ---- v transposes into [key, hd-pair] ----------------------
            for j in range(4):
                for b in range(NB):
                    tag = "zda" if b % 2 == 0 else "zdb"
                    zt = ps.tile(
                        [128, W], F32, tag=tag, bufs=1,
                        padded_shape=[128, 512], name=f"zt{j}_{b}",
                    )
                    tp = zt.bitcast(BF16)
                    nc.tensor.transpose(
                        tp[:, 0:128], vts[j][:, 128 * b : 128 * (b + 1)], id2[:]
                    )
                    nc.vector.tensor_copy(vaug[j][b][:], tp[:, 0:128])

            # ---- attention: per (half, head-pair), bank-granular PSUM --
            for t in range(2):
                for j in range(4):
                    qsl = slice(Q * j + QH * t, Q * j + QH * (t + 1))
                    zda = ps.tile(
                        [128, W], F32, tag="zda", bufs=1,
                        padded_shape=[128, 512], name=f"zda{t}{j}",
                    )
                    zdb = ps.tile(
                        [128, W], F32, tag="zdb", bufs=1,
                        padded_shape=[128, 512], name=f"zdb{t}{j}",
                    )
                    for b in range(NB):
                        nc.tensor.matmul(
                            zda[:, 128 * b : 128 * (b + 1)],
                            kts[j][0:64, 128 * b : 128 * (b + 1)],
                            qt[0:64, qsl], start=True, stop=True,
                        )
                        nc.tensor.matmul(
                            zdb[:, 128 * b : 128 * (b + 1)],
                            kts[j][64:128, 128 * b : 128 * (b + 1)],
                            qt[64:128, qsl], start=True, stop=True,
                        )
                    ee = estream.tile([128, W], BF16, tag="e", name=f"ee{t}{j}")
                    eo = estream.tile([128, W], BF16, tag="e", name=f"eo{t}{j}")
                    nc.scalar.activation(ee[:], zda[:], Exp)
                    nc.scalar.activation(eo[:], zdb[:], Exp)
                    nc.vector.tensor_mul(ee[:], ee[:], mt[:, W * t : W * (t + 1)])
                    nc.vector.tensor_mul(eo[:], eo[:], mt[:, W * t : W * (t + 1)])
                    uacc = ps.tile(
                        [HD, 2 * QH], F32, tag="uacc", bufs=2,
                        padded_shape=[HD, 512], name=f"ua{t}{j}",
                    )
                    dnp = ps.tile(
                        [HD, 2 * QH], F32, tag="dn", bufs=2,
                        padded_shape=[HD, 512], name=f"dn{t}{j}",
                    )
                    for b in range(NB):
                        nc.tensor.matmul(
                            uacc[:, 0:QH], vaug[j][b][:, 0:64],
                            ee[:, 128 * b : 128 * (b + 1)],
                            start=(b == 0), stop=(b == NB - 1),
                        )
                    for b in range(NB):
                        nc.tensor.matmul(
                            uacc[:, QH : 2 * QH], vaug[j][b][:, 64:128],
                            eo[:, 128 * b : 128 * (b + 1)],
                            start=(b == 0), stop=(b == NB - 1),
                        )
                    for b in range(NB):
                        nc.tensor.matmul(
                            dnp[:, 0:QH], ones[:, 0:64],
                            ee[:, 128 * b : 128 * (b + 1)],
                            start=(b == 0), stop=(b == NB - 1),
                        )
                    for b in range(NB):
                        nc.tensor.matmul(
                            dnp[:, QH : 2 * QH], ones[:, 0:64],
                            eo[:, 128 * b : 128 * (b + 1)],
                            start=(b == 0), stop=(b == NB - 1),
                        )
                    # per-pair normalization (overlaps later pairs)
                    nc.vector.reciprocal(
                        rcp[t][:, 2 * QH * j : 2 * QH * (j + 1)], dnp[:, 0 : 2 * QH]
                    )
                    nc.vector.tensor_mul(
                        un[t][:, 2 * QH * j : 2 * QH * (j + 1)],
                        uacc[:, 0 : 2 * QH],
                        rcp[t][:, 2 * QH * j : 2 * QH * (j + 1)],
                    )

            # ---- output projection per half ----------------------------
            for t in range(2):
                yp = ps.tile([128, 4 * QH], F32, tag="big", bufs=2, name=f"yp{t}")
                for g in range(4):
                    for h in range(H):
                        nc.tensor.matmul(
                            yp[:, QH * g : QH * (g + 1)],
                            wo[:, D * h + 128 * g : D * h + 128 * (g + 1)],
                            un[t][:, QH * h : QH * (h + 1)],
                            start=(h == 0),
                            stop=(h == H - 1),
                        )
                for g in range(4):
                    nc.scalar.activation(
                        y_sb[:, 4 * QH * t + QH * g : 4 * QH * t + QH * (g + 1)],
                        yp[:, QH * g : QH * (g + 1)],
                        Identity,
                        bias=bo[:, g : g + 1],
                    )
                eng = nc.sync if t == 0 else nc.scalar
                eng.dma_start(
                    y_d[:, 4 * QH * t : 4 * QH * (t + 1)],
                    y_sb[:, 4 * QH * t : 4 * QH * (t + 1)],
                )
    nc.compile()
    return nc


def _chunk128(a):
    """[D, N] -> [128, 4*N] with col = N*cc + n."""
    Dd, N = a.shape
    return np.ascontiguousarray(
        a.reshape(4, 128, N).transpose(1, 0, 2).reshape(128, 4 * N)
    )


def _prep_inputs(x, routes, W_qkv, b_qkv, W_out, b_out, perm, rank, w0s, W):
    NB = W // 128
    x2 = np.asarray(x, dtype=np.float32).reshape(S, D)
    routes = np.asarray(routes).astype(np.int64)
    Wq = np.asarray(W_qkv, dtype=np.float32)
    bqv = np.asarray(b_qkv, dtype=np.float32)
    Wo = np.asarray(W_out, dtype=np.float32)

    xl = np.concatenate([x2[0:1], x2[:-1]], axis=0)
    xr = np.concatenate([x2[1:], x2[-1:]], axis=0)
    xs = 0.5 * x2 + 0.25 * (xl + xr)

    wq_r = _chunk128(np.ascontiguousarray((Wq[:, 0:D] / 8.0))).astype(ml_dtypes.bfloat16)
    wkv_r = _chunk128(np.ascontiguousarray(Wq[:, D : 3 * D])).astype(ml_dtypes.bfloat16)
    # wo: [64, 8*512], col = 512*h + o, row p = channel 64h+p
    wo_r = np.ascontiguousarray(
        Wo.reshape(8, 64, D).transpose(1, 0, 2).reshape(64, 8 * D)
    ).astype(ml_dtypes.bfloat16)
    bq_r = np.ascontiguousarray((bqv[0:D] / 8.0).reshape(4, 128).T, dtype=np.float32)
    bkv_r = np.zeros((128, 8), dtype=np.float32)
    for j in range(4):
        bkv_r[:, j] = bqv[D + 128 * j : D + 128 * (j + 1)]
        bkv_r[:, 4 + j] = bqv[2 * D + 128 * j : 2 * D + 128 * (j + 1)]
    bo_r = np.ascontiguousarray(
        np.asarray(b_out, dtype=np.float32).reshape(4, 128).T, dtype=np.float32
    )
    id2 = np.eye(128, dtype=ml_dtypes.bfloat16)
    ones_r = np.ones((128, HD), dtype=ml_dtypes.bfloat16)

    in_maps = []
    for c in range(NCORES):
        qpos = perm[Q * c : Q * (c + 1)]
        wpos = perm[w0s[c] : w0s[c] + W]
        xq_r = _chunk128(np.ascontiguousarray(x2[qpos].T)).astype(ml_dtypes.bfloat16)
        xw_r = _chunk128(np.ascontiguousarray(xs[wpos].T)).astype(ml_dtypes.bfloat16)
        # mask [W, Q] -> layout [128, (2b+t)*256 : ...] duplicated per head parity
        mtc = np.zeros((W, Q), dtype=np.float32)
        wrank = rank[routes[qpos]] - w0s[c]
        np.add.at(
            mtc, (wrank.ravel(), np.repeat(np.arange(Q), routes.shape[1])), 1.0
        )
        mt_r = np.zeros((128, 2 * W), dtype=ml_dtypes.bfloat16)
        for t in range(2):
            for b in range(NB):
                mt_r[:, W * t + 128 * b : W * t + 128 * (b + 1)] = mtc[
                    128 * b : 128 * (b + 1), QH * t : QH * (t + 1)
                ]
        in_maps.append(
            {
                "xq": xq_r,
                "xw": xw_r,
                "wq": wq_r,
                "wkv": wkv_r,
                "wo": wo_r,
                "mt": mt_r,
                "bq": bq_r,
                "bkv": bkv_r,
                "bo": bo_r,
                "id2": id2,
                "ones": ones_r,
            }
        )
    return in_maps


def _run(nc, in_maps, **kw):
    return bass_utils.run_bass_kernel_spmd(nc, in_maps, list(range(NCORES)), **kw)


def kernel(x, routes, W_qkv, b_qkv, W_out, b_out, _timing=None):
    perm, rank, w0s, W = _plan(routes)
    if W not in _nc_cache:
        _nc_cache[W] = _build(W)
    nc = _nc_cache[W]
    in_maps = _prep_inputs(x, routes, W_qkv, b_qkv, W_out, b_out, perm, rank, w0s, W)
    r = _run(nc, in_maps)
    y = np.zeros((S, D), dtype=np.float32)
    for c in range(NCORES):
        yc = r.results[c]["y"]  # [128, 2*4*QH]
        qpos = perm[Q * c : Q * (c + 1)]
        # y[qpos[QH*t + i], 128g + p] = yc[p, 512t + QH*g + i]
        blk = yc.reshape(128, 2, 4, QH).transpose(1, 3, 2, 0).reshape(Q, D)
        y[qpos] = blk
    if _timing is not None:
        _timing["r1"] = r
        _timing["in1"] = in_maps
        _timing["nc"] = nc
    return y.reshape(1, S, D).astype(np.float32)


# revision 10
# speedup vs baseline: 2.0497x; 1.0520x over previous
"""CantorAttention Trainium2 kernel (8 NeuronCores) — single-NEFF block-sparse.

Key ideas
---------
1. Sorting positions by Cantor coordinate makes each query's 64 routed keys
   fall in a narrow contiguous window of the sorted order (k-nearest
   neighbours in 1D are contiguous after sorting). Sharding the *sorted*
   sequence 8 ways gives each core 256 queries whose keys live in a 384-wide
   rank window -> 3 key blocks of 128 instead of 16.
2. The +-1 position smoothing commutes with the (linear) qkv projection, so
   the host feeds x-smoothed (x~) gathered in rank order and the device
   projects it directly into smoothed k~/v~ — no on-device gather.
3. Softmax over routed slots == dense masked softmax over the window with
   multiplicity mask M, evaluated as exp(scores) * M; denominators come from
   an all-ones-stationary matmul replicated across 64 partitions so the
   reciprocal + normalization run as plain aligned [64, .] vector ops.
4. Each core computes *all heads* for its query slice, so the output
   projection is local: one NEFF, no cross-core exchange, no second launch.

Hardware rules baked in: one OPEN PSUM accumulation group per 2KB bank at a
time (concurrent groups in one bank corrupt/crash), row-group-concurrent
matmul pairs write different banks, every PSUM tag slot is padded to a full
bank.
"""
import sys

sys.path.insert(0, "/opt/trn_rl_repo")

import numpy as np
import ml_dtypes

import concourse.bass as bass
import concourse.bacc as bacc
import concourse.mybir as mybir
from concourse import tile
from concourse import bass_utils

BF16 = mybir.dt.bfloat16
F32 = mybir.dt.float32
Exp = mybir.ActivationFunctionType.Exp
Copy = mybir.ActivationFunctionType.Copy
Identity = mybir.ActivationFunctionType.Identity

S = 2048
D = 512
H = 8
HD = 64
NCORES = 8
Q = S // NCORES  # 256 queries per core

_nc_cache = {}


def _cantor_coords(seq_len, depth=8):
    x = np.arange(seq_len, dtype=np.float64) / max(1, seq_len - 1)
    x = np.clip(x, 1e-06, 1.0 - 1e-06)
    c = np.zeros_like(x)
    factor = 0.5
    for _ in range(depth):
        xs = x * 3.0
        digit = xs.astype(np.int64)
        x = xs - digit
        c = c + (digit == 2).astype(np.float64) * factor
        factor *= 0.5
    return np.clip(c, 0.0, 1.0)


def _candidate_orders(routes):
    Sl = routes.shape[0]
    yield np.argsort(_cantor_coords(Sl), kind="stable")
    try:
        import scipy.sparse as sp
        from scipy.sparse.csgraph import reverse_cuthill_mckee

        rows = np.repeat(np.arange(Sl), routes.shape[1])
        cols = np.asarray(routes).ravel()
        A = sp.coo_matrix(
            (np.ones(rows.size, dtype=np.float32), (rows, cols)), shape=(Sl, Sl)
        ).tocsr()
        yield np.asarray(reverse_cuthill_mckee(A + A.T)).astype(np.int64)
    except Exception:
        pass
    yield np.arange(Sl)


def _plan(routes):
    """Pick ordering + window width. Returns (perm, rank, w0s, W)."""
    routes = np.asarray(routes).astype(np.int64)
    best = None
    for perm in _candidate_orders(routes):
        rank = np.empty(S, dtype=np.int64)
        rank[perm] = np.arange(S)
        lo = np.empty(NCORES, dtype=np.int64)
        hi = np.empty(NCORES, dtype=np.int64)
        for c in range(NCORES):
            kr = rank[routes[perm[Q * c : Q * (c + 1)]]]
            lo[c], hi[c] = kr.min(), kr.max()
        width = int((hi - lo + 1).max())
        if best is None or width < best[0]:
            best = (width, perm, rank, lo)
        if width <= 384:
            break
    width, perm, rank, lo = best
    W = 384
    while W < width:
        W += 128
    W = min(W, S)
    w0s = np.minimum(np.maximum(lo, 0), S - W)
    return perm, rank, w0s, W


def _build(W):
    """Single-NEFF kernel for window width W (multiple of 128)."""
    NB = W // 128
    nc = bacc.Bacc("TRN2", target_bir_lowering=False, debug=False, num_devices=NCORES)
    xq_d = nc.dram_tensor("xq", [128, 4 * Q], BF16, kind="ExternalInput").ap()
    xw_d = nc.dram_tensor("xw", [128, 4 * W], BF16, kind="ExternalInput").ap()
    wq_d = nc.dram_tensor("wq", [128, 4 * D], BF16, kind="ExternalInput").ap()
    wkv_d = nc.dram_tensor("wkv", [128, 4 * 2 * D], BF16, kind="ExternalInput").ap()
    wo_d = nc.dram_tensor("wo", [HD + 1, 8 * D], BF16, kind="ExternalInput").ap()
    mt_d = nc.dram_tensor("mt", [128, NB * Q], BF16, kind="ExternalInput").ap()
    bq_d = nc.dram_tensor("bq", [128, 4], F32, kind="ExternalInput").ap()
    bkv_d = nc.dram_tensor("bkv", [128, 8], F32, kind="ExternalInput").ap()
    id2_d = nc.dram_tensor("id2", [128, 128], BF16, kind="ExternalInput").ap()
    ones_d = nc.dram_tensor("ones", [128, HD], BF16, kind="ExternalInput").ap()
    y_d = nc.dram_tensor("y", [128, 2 * D], F32, kind="ExternalOutput").ap()

    with tile.TileContext(nc) as tc:
        with (
            tc.tile_pool(name="const", bufs=1) as const,
            tc.tile_pool(name="work", bufs=1) as work,
            tc.tile_pool(name="estream", bufs=8) as estream,
            tc.tile_pool(name="ps", bufs=1, space="PSUM") as ps,
        ):
            # ---- constants / inputs ------------------------------------
            xq = const.tile([128, 4 * Q], BF16)
            xw = const.tile([128, 4 * W], BF16)
            wq = const.tile([128, 4 * D], BF16)
            wkv = const.tile([128, 4 * 2 * D], BF16)
            wo = const.tile([HD + 1, 8 * D], BF16)
            mt = const.tile([128, NB * Q], BF16)
            bq = const.tile([128, 4], F32)
            bkv = const.tile([128, 8], F32)
            id2 = const.tile([128, 128], BF16)
            ones = const.tile([128, HD], BF16)

            # critical path: qproj needs wq+xq, then kvproj needs wkv+xw.
            for c in range(4):
                nc.sync.dma_start(wq[:, D * c : D * (c + 1)], wq_d[:, D * c : D * (c + 1)])
                nc.scalar.dma_start(xq[:, Q * c : Q * (c + 1)], xq_d[:, Q * c : Q * (c + 1)])
            for c in range(4):
                nc.sync.dma_start(
                    wkv[:, 2 * D * c : 2 * D * (c + 1)], wkv_d[:, 2 * D * c : 2 * D * (c + 1)]
                )
                nc.scalar.dma_start(xw[:, W * c : W * (c + 1)], xw_d[:, W * c : W * (c + 1)])
            nc.gpsimd.dma_start(bq[:], bq_d[:])
            nc.gpsimd.dma_start(bkv[:], bkv_d[:])
            nc.gpsimd.dma_start(id2[:], id2_d[:])
            nc.gpsimd.dma_start(ones[:], ones_d[:])
            nc.scalar.dma_start(mt[:], mt_d[:])
            nc.gpsimd.dma_start(wo[:], wo_d[:])

            # ---- persistent SBUF ---------------------------------------
            qt = work.tile([128, 4 * Q], BF16)  # pair j: rows 0:64=q2j, 64:128=q2j+1
            kts = [work.tile([128, W], BF16, name=f"kt{j}") for j in range(4)]
            vts = [work.tile([128, W], BF16, name=f"vt{j}") for j in range(4)]
            vaug = [
                [work.tile([128, 128], BF16, name=f"va{j}_{b}") for b in range(NB)]
                for j in range(4)
            ]
            un = work.tile([HD + 1, H * Q], BF16)  # row 64 = ones (bias fold)
            rcp = work.tile([HD, H * Q], F32)
            y_sb = work.tile([128, 2 * D], F32)
            nc.gpsimd.memset(un[64:65, :], 1.0)

            # ---- projections -------------------------------------------
            for j in range(4):
                qp = ps.tile(
                    [128, Q], F32, tag="big", bufs=2, padded_shape=[128, 512],
                    name=f"qp{j}",
                )
                for c in range(4):
                    nc.tensor.matmul(
                        qp[:],
                        wq[:, D * c + 128 * j : D * c + 128 * (j + 1)],
                        xq[:, Q * c : Q * (c + 1)],
                        start=(c == 0),
                        stop=(c == 3),
                    )
                nc.scalar.activation(
                    qt[:, Q * j : Q * (j + 1)], qp[:], Identity, bias=bq[:, j : j + 1]
                )
            for j in range(4):
                for v, dst in ((0, kts[j]), (1, vts[j])):
                    col = 128 * j if v == 0 else D + 128 * j
                    kp = ps.tile(
                        [128, W], F32, tag="big", bufs=2, padded_shape=[128, 512],
                        name=f"kp{2 * j + v}",
                    )
                    for c in range(4):
                        nc.tensor.matmul(
                            kp[:],
                            wkv[:, 2 * D * c + col : 2 * D * c + col + 128],
                            xw[:, W * c : W * (c + 1)],
                            start=(c == 0),
                            stop=(c == 3),
                        )
                    bcol = j if v == 0 else 4 + j
                    nc.scalar.activation(
                        dst[:], kp[:], Identity, bias=bkv[:, bcol : bcol + 1]
                    )

            # ---- v transposes into [key, hd-pair] ----------------------
            for j in range(4):
                for b in range(NB):
                    tag = "zda" if b % 2 == 0 else "zdb"
                    zt = ps.tile(
                        [128, Q], F32, tag=tag, bufs=1, padded_shape=[128, 512],
                        name=f"zt{j}_{b}",
                    )
                    tp = zt.bitcast(BF16)
                    nc.tensor.transpose(
                        tp[:, 0:128], vts[j][:, 128 * b : 128 * (b + 1)], id2[:]
                    )
                    nc.vector.tensor_copy(vaug[j][b][:], tp[:, 0:128])

            # ---- attention: per head-pair, bank-granular PSUM ----------
            for j in range(4):
                ees, eos = [], []
                for b in range(NB):
                    zda = ps.tile(
                        [128, Q], F32, tag="zda", bufs=1, padded_shape=[128, 512],
                        name=f"zda{j}{b}",
                    )
                    zdb = ps.tile(
                        [128, Q], F32, tag="zdb", bufs=1, padded_shape=[128, 512],
                        name=f"zdb{j}{b}",
                    )
                    nc.tensor.matmul(
                        zda[:],
                        kts[j][0:64, 128 * b : 128 * (b + 1)],
                        qt[0:64, Q * j : Q * (j + 1)],
                        start=True,
                        stop=True,
                    )
                    nc.tensor.matmul(
                        zdb[:],
                        kts[j][64:128, 128 * b : 128 * (b + 1)],
                        qt[64:128, Q * j : Q * (j + 1)],
                        start=True,
                        stop=True,
                    )
                    ee = estream.tile([128, Q], BF16, tag="e", name=f"ee{j}{b}")
                    eo = estream.tile([128, Q], BF16, tag="e", name=f"eo{j}{b}")
                    nc.scalar.activation(ee[:], zda[:], Exp)
                    nc.scalar.activation(eo[:], zdb[:], Exp)
                    nc.vector.tensor_mul(ee[:], ee[:], mt[:, Q * b : Q * (b + 1)])
                    nc.gpsimd.tensor_mul(eo[:], eo[:], mt[:, Q * b : Q * (b + 1)])
                    ees.append(ee)
                    eos.append(eo)
                uacc = ps.tile(
                    [HD, 2 * Q], F32, tag="uacc", bufs=2, name=f"ua{j}"
                )
                dnp = ps.tile([HD, 2 * Q], F32, tag="dn", bufs=2, name=f"dn{j}")
                for b in range(NB):
                    nc.tensor.matmul(
                        uacc[:, 0:Q], vaug[j][b][:, 0:64], ees[b][:],
                        start=(b == 0), stop=(b == NB - 1),
                    )
                for b in range(NB):
                    nc.tensor.matmul(
                        uacc[:, Q : 2 * Q], vaug[j][b][:, 64:128], eos[b][:],
                        start=(b == 0), stop=(b == NB - 1),
                    )
                for b in range(NB):
                    nc.tensor.matmul(
                        dnp[:, 0:Q], ones[:, 0:64], ees[b][:],
                        start=(b == 0), stop=(b == NB - 1),
                    )
                for b in range(NB):
                    nc.tensor.matmul(
                        dnp[:, Q : 2 * Q], ones[:, 0:64], eos[b][:],
                        start=(b == 0), stop=(b == NB - 1),
                    )
                # normalization for pair j (overlaps later pairs)
                nc.vector.reciprocal_approx_fast(
                    rcp[:, 2 * Q * j : 2 * Q * (j + 1)], dnp[:]
                )
                nc.vector.tensor_mul(
                    un[0:64, 2 * Q * j : 2 * Q * (j + 1)],
                    uacc[:],
                    rcp[:, 2 * Q * j : 2 * Q * (j + 1)],
                )

            # ---- output projection (stationary = un, bias via ones row)
            for t in range(2):
                yp = ps.tile(
                    [128, D], F32, tag="big", bufs=2, name=f"yp{t}"
                )
                for h in range(H):
                    nc.tensor.matmul(
                        yp[:],
                        un[:, Q * h + 128 * t : Q * h + 128 * (t + 1)],
                        wo[:, D * h : D * (h + 1)],
                        start=(h == 0),
                        stop=(h == H - 1),
                    )
                nc.scalar.activation(y_sb[:, D * t : D * (t + 1)], yp[:], Copy)
                eng = nc.sync if t == 0 else nc.scalar
                eng.dma_start(y_d[:, D * t : D * (t + 1)], y_sb[:, D * t : D * (t + 1)])
    nc.compile()
    return nc


def _chunk128(a):
    """[D, N] -> [128, 4*N] with col = N*cc + n."""
    Dd, N = a.shape
    return np.ascontiguousarray(
        a.reshape(4, 128, N).transpose(1, 0, 2).reshape(128, 4 * N)
    )


def _prep_inputs(x, routes, W_qkv, b_qkv, W_out, b_out, perm, rank, w0s, W):
    NB = W // 128
    x2 = np.asarray(x, dtype=np.float32).reshape(S, D)
    routes = np.asarray(routes).astype(np.int64)
    Wq = np.asarray(W_qkv, dtype=np.float32)
    bqv = np.asarray(b_qkv, dtype=np.float32)
    Wo = np.asarray(W_out, dtype=np.float32)

    xl = np.concatenate([x2[0:1], x2[:-1]], axis=0)
    xr = np.concatenate([x2[1:], x2[-1:]], axis=0)
    xs = 0.5 * x2 + 0.25 * (xl + xr)

    wq_r = _chunk128(np.ascontiguousarray((Wq[:, 0:D] / 8.0))).astype(ml_dtypes.bfloat16)
    wkv_r = _chunk128(np.ascontiguousarray(Wq[:, D : 3 * D])).astype(ml_dtypes.bfloat16)
    # wo: [65, 8*512], col = 512*h + o; row p<64 = W_out[64h+p, o]; row 64 = bias (h=0)
    wo_r = np.zeros((HD + 1, 8 * D), dtype=ml_dtypes.bfloat16)
    wo_r[0:64] = Wo.reshape(8, 64, D).transpose(1, 0, 2).reshape(64, 8 * D).astype(
        ml_dtypes.bfloat16
    )
    wo_r[64, 0:D] = np.asarray(b_out, dtype=np.float32).astype(ml_dtypes.bfloat16)
    bq_r = np.ascontiguousarray((bqv[0:D] / 8.0).reshape(4, 128).T, dtype=np.float32)
    bkv_r = np.zeros((128, 8), dtype=np.float32)
    for j in range(4):
        bkv_r[:, j] = bqv[D + 128 * j : D + 128 * (j + 1)]
        bkv_r[:, 4 + j] = bqv[2 * D + 128 * j : 2 * D + 128 * (j + 1)]
    id2 = np.eye(128, dtype=ml_dtypes.bfloat16)
    ones_r = np.ones((128, HD), dtype=ml_dtypes.bfloat16)

    in_maps = []
    for c in range(NCORES):
        qpos = perm[Q * c : Q * (c + 1)]
        wpos = perm[w0s[c] : w0s[c] + W]
        xq_r = _chunk128(np.ascontiguousarray(x2[qpos].T)).astype(ml_dtypes.bfloat16)
        xw_r = _chunk128(np.ascontiguousarray(xs[wpos].T)).astype(ml_dtypes.bfloat16)
        mtc = np.zeros((W, Q), dtype=np.float32)
        wrank = rank[routes[qpos]] - w0s[c]
        np.add.at(
            mtc, (wrank.ravel(), np.repeat(np.arange(Q), routes.shape[1])), 1.0
        )
        mt_r = np.zeros((128, NB * Q), dtype=ml_dtypes.bfloat16)
        for b in range(NB):
            mt_r[:, Q * b : Q * (b + 1)] = mtc[128 * b : 128 * (b + 1), :]
        in_maps.append(
            {
                "xq": xq_r,
                "xw": xw_r,
                "wq": wq_r,
                "wkv": wkv_r,
                "wo": wo_r,
                "mt": mt_r,
                "bq": bq_r,
                "bkv": bkv_r,
                "id2": id2,
                "ones": ones_r,
            }
        )
    return in_maps


def _run(nc, in_maps, **kw):
    return bass_utils.run_bass_kernel_spmd(nc, in_maps, list(range(NCORES)), **kw)


def kernel(x, routes, W_qkv, b_qkv, W_out, b_out, _timing=None):
    perm, rank, w0s, W = _plan(routes)
    if W not in _nc_cache:
        _nc_cache[W] = _build(W)
    nc = _nc_cache[W]
    in_maps = _prep_inputs(x, routes, W_qkv, b_qkv, W_out, b_out, perm, rank, w0s, W)
    r = _run(nc, in_maps)
    y = np.zeros((S, D), dtype=np.float32)
    for c in range(NCORES):
        yc = r.results[c]["y"]  # [128, 2*D]: y[qpos[128t+p], o] = yc[p, 512t+o]
        qpos = perm[Q * c : Q * (c + 1)]
        y[qpos[0:128]] = yc[:, 0:D]
        y[qpos[128:256]] = yc[:, D : 2 * D]
    if _timing is not None:
        _timing["r1"] = r
        _timing["in1"] = in_maps
        _timing["nc"] = nc
    return y.reshape(1, S, D).astype(np.float32)


# revision 13
# speedup vs baseline: 2.4663x; 1.2033x over previous
"""CantorAttention Trainium2 kernel (8 NeuronCores) — single-NEFF block-sparse.

Key ideas
---------
1. Sorting positions by Cantor coordinate makes each query's 64 routed keys
   fall in a narrow contiguous window of the sorted order (k-nearest
   neighbours in 1D are contiguous after sorting). Sharding the *sorted*
   sequence 8 ways gives each core 256 queries whose keys live in a 384-wide
   rank window -> 3 key blocks of 128 instead of 16.
2. The +-1 position smoothing commutes with the (linear) qkv projection, so
   the host feeds x-smoothed (x~) gathered in rank order and the device
   projects it directly into smoothed k~/v~ — no on-device gather.
3. Softmax over routed slots == dense masked softmax over the window with
   multiplicity mask M, evaluated as exp(scores) * M; denominators come from
   an all-ones-stationary matmul replicated across 64 partitions so the
   reciprocal + normalization run as plain aligned [64, .] vector ops.
4. Each core computes *all heads* for its query slice, so the output
   projection is local: one NEFF, no cross-core exchange, no second launch.

Hardware rules baked in: one OPEN PSUM accumulation group per 2KB bank at a
time (concurrent groups in one bank corrupt/crash), row-group-concurrent
matmul pairs write different banks, every PSUM tag slot is padded to a full
bank.
"""
import sys

sys.path.insert(0, "/opt/trn_rl_repo")

import numpy as np
import ml_dtypes

import concourse.bass as bass
import concourse.bacc as bacc
import concourse.mybir as mybir
from concourse import tile
from concourse import bass_utils

BF16 = mybir.dt.bfloat16
F32 = mybir.dt.float32
Exp = mybir.ActivationFunctionType.Exp
Copy = mybir.ActivationFunctionType.Copy
Identity = mybir.ActivationFunctionType.Identity

S = 2048
D = 512
H = 8
HD = 64
NCORES = 8
Q = S // NCORES  # 256 queries per core

_nc_cache = {}


def _cantor_coords(seq_len, depth=8):
    x = np.arange(seq_len, dtype=np.float64) / max(1, seq_len - 1)
    x = np.clip(x, 1e-06, 1.0 - 1e-06)
    c = np.zeros_like(x)
    factor = 0.5
    for _ in range(depth):
        xs = x * 3.0
        digit = xs.astype(np.int64)
        x = xs - digit
        c = c + (digit == 2).astype(np.float64) * factor
        factor *= 0.5
    return np.clip(c, 0.0, 1.0)


def _candidate_orders(routes):
    Sl = routes.shape[0]
    yield np.argsort(_cantor_coords(Sl), kind="stable")
    try:
        import scipy.sparse as sp
        from scipy.sparse.csgraph import reverse_cuthill_mckee

        rows = np.repeat(np.arange(Sl), routes.shape[1])
        cols = np.asarray(routes).ravel()
        A = sp.coo_matrix(
            (np.ones(rows.size, dtype=np.float32), (rows, cols)), shape=(Sl, Sl)
        ).tocsr()
        yield np.asarray(reverse_cuthill_mckee(A + A.T)).astype(np.int64)
    except Exception:
        pass
    yield np.arange(Sl)


def _plan(routes):
    """Pick ordering + window width. Returns (perm, rank, w0s, W)."""
    routes = np.asarray(routes).astype(np.int64)
    best = None
    for perm in _candidate_orders(routes):
        rank = np.empty(S, dtype=np.int64)
        rank[perm] = np.arange(S)
        lo = np.empty(NCORES, dtype=np.int64)
        hi = np.empty(NCORES, dtype=np.int64)
        for c in range(NCORES):
            kr = rank[routes[perm[Q * c : Q * (c + 1)]]]
            lo[c], hi[c] = kr.min(), kr.max()
        width = int((hi - lo + 1).max())
        if best is None or width < best[0]:
            best = (width, perm, rank, lo)
        if width <= 384:
            break
    width, perm, rank, lo = best
    W = 384
    while W < width:
        W += 128
    W = min(W, S)
    w0s = np.minimum(np.maximum(lo, 0), S - W)
    return perm, rank, w0s, W


def _build(W):
    """Single-NEFF kernel for window width W (multiple of 128)."""
    NB = W // 128
    nc = bacc.Bacc("TRN2", target_bir_lowering=False, debug=False, num_devices=NCORES)
    xq_d = nc.dram_tensor("xq", [128, 4 * Q], BF16, kind="ExternalInput").ap()
    xw_d = nc.dram_tensor("xw", [128, 4 * W], BF16, kind="ExternalInput").ap()
    wq_d = nc.dram_tensor("wq", [128, 4 * D], BF16, kind="ExternalInput").ap()
    wkv_d = nc.dram_tensor("wkv", [128, 4 * 2 * D], BF16, kind="ExternalInput").ap()
    wo_d = nc.dram_tensor("wo", [HD + 1, 8 * D], BF16, kind="ExternalInput").ap()
    mt_d = nc.dram_tensor("mt", [128, NB * Q], BF16, kind="ExternalInput").ap()
    bq_d = nc.dram_tensor("bq", [128, 4], F32, kind="ExternalInput").ap()
    bkv_d = nc.dram_tensor("bkv", [128, 8], F32, kind="ExternalInput").ap()
    id2_d = nc.dram_tensor("id2", [128, 128], BF16, kind="ExternalInput").ap()
    ones_d = nc.dram_tensor("ones", [128, HD], BF16, kind="ExternalInput").ap()
    y_d = nc.dram_tensor("y", [128, 2 * D], F32, kind="ExternalOutput").ap()

    with tile.TileContext(nc) as tc:
        with (
            tc.tile_pool(name="const", bufs=1) as const,
            tc.tile_pool(name="work", bufs=1) as work,
            tc.tile_pool(name="estream", bufs=4) as estream,
            tc.tile_pool(name="ps", bufs=1, space="PSUM") as ps,
        ):
            # ---- constants / inputs ------------------------------------
            xq = const.tile([128, 4 * Q], BF16)
            xw = const.tile([128, 4 * W], BF16)
            wq = const.tile([128, 4 * D], BF16)
            wkv = const.tile([128, 4 * 2 * D], BF16)
            wo = const.tile([HD + 1, 8 * D], BF16)
            mt = const.tile([128, NB * Q], BF16)
            bq = const.tile([128, 4], F32)
            bkv = const.tile([128, 8], F32)
            id2 = const.tile([128, 128], BF16)
            ones = const.tile([128, HD], BF16)

            # DMA round-robin over all 5 engine queues, in dependency order:
            # qproj deps (xq, wq) first, then kv deps, then mask/consts/wo.
            qs = [nc.sync, nc.scalar, nc.gpsimd]
            loads = []
            for c in range(4):
                loads.append((xq[:, Q * c : Q * (c + 1)], xq_d[:, Q * c : Q * (c + 1)]))
                loads.append((wq[:, D * c : D * (c + 1)], wq_d[:, D * c : D * (c + 1)]))
            for c in range(4):
                loads.append((xw[:, W * c : W * (c + 1)], xw_d[:, W * c : W * (c + 1)]))
                loads.append(
                    (wkv[:, 2 * D * c : 2 * D * c + D], wkv_d[:, 2 * D * c : 2 * D * c + D])
                )
                loads.append(
                    (
                        wkv[:, 2 * D * c + D : 2 * D * (c + 1)],
                        wkv_d[:, 2 * D * c + D : 2 * D * (c + 1)],
                    )
                )
            loads.append((id2[:], id2_d[:]))
            loads.append((ones[:], ones_d[:]))
            loads.append((bq[:], bq_d[:]))
            loads.append((bkv[:], bkv_d[:]))
            for b in range(NB):
                loads.append((mt[:, Q * b : Q * (b + 1)], mt_d[:, Q * b : Q * (b + 1)]))
            for c in range(4):
                loads.append(
                    (wo[:, 2 * D * c : 2 * D * (c + 1)], wo_d[:, 2 * D * c : 2 * D * (c + 1)])
                )
            for i, (dst, srcap) in enumerate(loads):
                qs[i % 3].dma_start(dst, srcap)

            # ---- persistent SBUF ---------------------------------------
            qt = work.tile([128, 4 * Q], BF16)  # pair j: rows 0:64=q2j, 64:128=q2j+1
            kts = [work.tile([128, W], BF16, name=f"kt{j}") for j in range(4)]
            vts = [work.tile([128, W], BF16, name=f"vt{j}") for j in range(4)]
            vaug = [
                [work.tile([128, 128], BF16, name=f"va{j}_{b}") for b in range(NB)]
                for j in range(4)
            ]
            un = work.tile([HD + 1, H * Q], BF16)  # row 64 = ones (bias fold)
            rcp = work.tile([HD, H * Q], F32)
            y_sb = work.tile([128, 2 * D], F32)
            nc.gpsimd.memset(un[64:65, :], 1.0)

            # ---- projections -------------------------------------------
            for j in range(4):
                qp = ps.tile(
                    [128, Q], F32, tag="big", bufs=2, padded_shape=[128, 512],
                    name=f"qp{j}",
                )
                for c in range(4):
                    nc.tensor.matmul(
                        qp[:],
                        wq[:, D * c + 128 * j : D * c + 128 * (j + 1)],
                        xq[:, Q * c : Q * (c + 1)],
                        start=(c == 0),
                        stop=(c == 3),
                    )
                nc.scalar.activation(
                    qt[:, Q * j : Q * (j + 1)], qp[:], Identity, bias=bq[:, j : j + 1]
                )
            for j in range(4):
                for v, dst in ((0, kts[j]), (1, vts[j])):
                    col = 128 * j if v == 0 else D + 128 * j
                    kp = ps.tile(
                        [128, W], F32, tag="big", bufs=2, padded_shape=[128, 512],
                        name=f"kp{2 * j + v}",
                    )
                    for c in range(4):
                        nc.tensor.matmul(
                            kp[:],
                            wkv[:, 2 * D * c + col : 2 * D * c + col + 128],
                            xw[:, W * c : W * (c + 1)],
                            start=(c == 0),
                            stop=(c == 3),
                        )
                    bcol = j if v == 0 else 4 + j
                    nc.scalar.activation(
                        dst[:], kp[:], Identity, bias=bkv[:, bcol : bcol + 1]
                    )

            # ---- v transposes into [key, hd-pair] ----------------------
            for j in range(4):
                for b in range(NB):
                    tag = "zda" if b % 2 == 0 else "zdb"
                    zt = ps.tile(
                        [128, NB * Q], F32, tag=tag, bufs=1, padded_shape=[128, 1024],
                        name=f"zt{j}_{b}",
                    )
                    tp = zt.bitcast(BF16)
                    nc.tensor.transpose(
                        tp[:, 0:128], vts[j][:, 128 * b : 128 * (b + 1)], id2[:]
                    )
                    nc.vector.tensor_copy(vaug[j][b][:], tp[:, 0:128])

            # ---- attention: per head-pair, bank-granular PSUM ----------
            for j in range(4):
                zda = ps.tile(
                    [128, NB * Q], F32, tag="zda", bufs=1, padded_shape=[128, 1024],
                    name=f"zda{j}",
                )
                zdb = ps.tile(
                    [128, NB * Q], F32, tag="zdb", bufs=1, padded_shape=[128, 1024],
                    name=f"zdb{j}",
                )
                for b in range(NB):
                    nc.tensor.matmul(
                        zda[:, Q * b : Q * (b + 1)],
                        kts[j][0:64, 128 * b : 128 * (b + 1)],
                        qt[0:64, Q * j : Q * (j + 1)],
                        start=True,
                        stop=True,
                    )
                    nc.tensor.matmul(
                        zdb[:, Q * b : Q * (b + 1)],
                        kts[j][64:128, 128 * b : 128 * (b + 1)],
                        qt[64:128, Q * j : Q * (j + 1)],
                        start=True,
                        stop=True,
                    )
                ee = estream.tile([128, NB * Q], BF16, tag="e", name=f"ee{j}")
                eo = estream.tile([128, NB * Q], BF16, tag="e", name=f"eo{j}")
                nc.scalar.activation(ee[:], zda[:], Exp)
                nc.scalar.activation(eo[:], zdb[:], Exp)
                nc.vector.tensor_mul(ee[:], ee[:], mt[:])
                nc.vector.tensor_mul(eo[:], eo[:], mt[:])
                uacc = ps.tile([HD, 2 * Q], F32, tag="uacc", bufs=1, name=f"ua{j}")
                dnp = ps.tile([HD, 2 * Q], F32, tag="dn", bufs=1, name=f"dn{j}")
                for b in range(NB):
                    nc.tensor.matmul(
                        uacc[:, 0:Q], vaug[j][b][:, 0:64],
                        ee[:, Q * b : Q * (b + 1)],
                        start=(b == 0), stop=(b == NB - 1),
                    )
                for b in range(NB):
                    nc.tensor.matmul(
                        uacc[:, Q : 2 * Q], vaug[j][b][:, 64:128],
                        eo[:, Q * b : Q * (b + 1)],
                        start=(b == 0), stop=(b == NB - 1),
                    )
                for b in range(NB):
                    nc.tensor.matmul(
                        dnp[:, 0:Q], ones[:, 0:64], ee[:, Q * b : Q * (b + 1)],
                        start=(b == 0), stop=(b == NB - 1),
                    )
                for b in range(NB):
                    nc.tensor.matmul(
                        dnp[:, Q : 2 * Q], ones[:, 0:64], eo[:, Q * b : Q * (b + 1)],
                        start=(b == 0), stop=(b == NB - 1),
                    )
                # normalization for pair j (overlaps later pairs)
                nc.vector.reciprocal_approx_fast(
                    rcp[:, 2 * Q * j : 2 * Q * (j + 1)], dnp[:]
                )
                nc.vector.tensor_mul(
                    un[0:64, 2 * Q * j : 2 * Q * (j + 1)],
                    uacc[:],
                    rcp[:, 2 * Q * j : 2 * Q * (j + 1)],
                )

            # ---- output projection (stationary = un, bias via ones row)
            for t in range(2):
                yp = ps.tile(
                    [128, D], F32, tag="big", bufs=2, name=f"yp{t}"
                )
                for h in range(H):
                    nc.tensor.matmul(
                        yp[:],
                        un[:, Q * h + 128 * t : Q * h + 128 * (t + 1)],
                        wo[:, D * h : D * (h + 1)],
                        start=(h == 0),
                        stop=(h == H - 1),
                    )
                nc.scalar.activation(y_sb[:, D * t : D * (t + 1)], yp[:], Copy)
                eng = nc.sync if t == 0 else nc.scalar
                eng.dma_start(y_d[:, D * t : D * (t + 1)], y_sb[:, D * t : D * (t + 1)])
    nc.compile()
    return nc


def _chunk128(a):
    """[D, N] -> [128, 4*N] with col = N*cc + n."""
    Dd, N = a.shape
    return np.ascontiguousarray(
        a.reshape(4, 128, N).transpose(1, 0, 2).reshape(128, 4 * N)
    )


def _prep_inputs(x, routes, W_qkv, b_qkv, W_out, b_out, perm, rank, w0s, W):
    NB = W // 128
    x2 = np.asarray(x, dtype=np.float32).reshape(S, D)
    routes = np.asarray(routes).astype(np.int64)
    Wq = np.asarray(W_qkv, dtype=np.float32)
    bqv = np.asarray(b_qkv, dtype=np.float32)
    Wo = np.asarray(W_out, dtype=np.float32)

    xl = np.concatenate([x2[0:1], x2[:-1]], axis=0)
    xr = np.concatenate([x2[1:], x2[-1:]], axis=0)
    xs = 0.5 * x2 + 0.25 * (xl + xr)

    wq_r = _chunk128(np.ascontiguousarray((Wq[:, 0:D] / 8.0))).astype(ml_dtypes.bfloat16)
    wkv_r = _chunk128(np.ascontiguousarray(Wq[:, D : 3 * D])).astype(ml_dtypes.bfloat16)
    # wo: [65, 8*512], col = 512*h + o; row p<64 = W_out[64h+p, o]; row 64 = bias (h=0)
    wo_r = np.zeros((HD + 1, 8 * D), dtype=ml_dtypes.bfloat16)
    wo_r[0:64] = Wo.reshape(8, 64, D).transpose(1, 0, 2).reshape(64, 8 * D).astype(
        ml_dtypes.bfloat16
    )
    wo_r[64, 0:D] = np.asarray(b_out, dtype=np.float32).astype(ml_dtypes.bfloat16)
    bq_r = np.ascontiguousarray((bqv[0:D] / 8.0).reshape(4, 128).T, dtype=np.float32)
    bkv_r = np.zeros((128, 8), dtype=np.float32)
    for j in range(4):
        bkv_r[:, j] = bqv[D + 128 * j : D + 128 * (j + 1)]
        bkv_r[:, 4 + j] = bqv[2 * D + 128 * j : 2 * D + 128 * (j + 1)]
    id2 = np.eye(128, dtype=ml_dtypes.bfloat16)
    ones_r = np.ones((128, HD), dtype=ml_dtypes.bfloat16)

    in_maps = []
    for c in range(NCORES):
        qpos = perm[Q * c : Q * (c + 1)]
        wpos = perm[w0s[c] : w0s[c] + W]
        xq_r = _chunk128(np.ascontiguousarray(x2[qpos].T)).astype(ml_dtypes.bfloat16)
        xw_r = _chunk128(np.ascontiguousarray(xs[wpos].T)).astype(ml_dtypes.bfloat16)
        mtc = np.zeros((W, Q), dtype=np.float32)
        wrank = rank[routes[qpos]] - w0s[c]
        np.add.at(
            mtc, (wrank.ravel(), np.repeat(np.arange(Q), routes.shape[1])), 1.0
        )
        mt_r = np.zeros((128, NB * Q), dtype=ml_dtypes.bfloat16)
        for b in range(NB):
            mt_r[:, Q * b : Q * (b + 1)] = mtc[128 * b : 128 * (b + 1), :]
        in_maps.append(
            {
                "xq": xq_r,
                "xw": xw_r,
                "wq": wq_r,
                "wkv": wkv_r,
                "wo": wo_r,
                "mt": mt_r,
                "bq": bq_r,
                "bkv": bkv_r,
                "id2": id2,
                "ones": ones_r,
            }
        )
    return in_maps


def _run(nc, in_maps, **kw):
    return bass_utils.run_bass_kernel_spmd(nc, in_maps, list(range(NCORES)), **kw)


def kernel(x, routes, W_qkv, b_qkv, W_out, b_out, _timing=None):
    perm, rank, w0s, W = _plan(routes)
    if W not in _nc_cache:
        _nc_cache[W] = _build(W)
    nc = _nc_cache[W]
    in_maps = _prep_inputs(x, routes, W_qkv, b_qkv, W_out, b_out, perm, rank, w0s, W)
    r = _run(nc, in_maps)
    y = np.zeros((S, D), dtype=np.float32)
    for c in range(NCORES):
        yc = r.results[c]["y"]  # [128, 2*D]: y[qpos[128t+p], o] = yc[p, 512t+o]
        qpos = perm[Q * c : Q * (c + 1)]
        y[qpos[0:128]] = yc[:, 0:D]
        y[qpos[128:256]] = yc[:, D : 2 * D]
    if _timing is not None:
        _timing["r1"] = r
        _timing["in1"] = in_maps
        _timing["nc"] = nc
    return y.reshape(1, S, D).astype(np.float32)
